# revision 19
# baseline (speedup 1.0000x reference)
"""Trainium2 Bass kernel for 4-head spatial self-attention (f16 pipeline).

Computation (per batch b):
    xf = x[b] reshaped [C=256, n=4096]
    q/k/v = Wq/Wk/Wv @ xf            -> [128, n]   (rows = 4 heads x 32 dims)
    S_h   = (q_h^T k_h) * 32^-0.5    -> [n, n] per head
    P     = exp(S)   (softmax without max-subtraction: logits are O(6))
    A_h   = P_h^T-normalized @ v_h   -> [n, 32]
    out   = Wout @ A + bout          -> [C, n]

Sharding: 8 cores = 4 batches x 2 query-halves. Each core handles all 4 heads
for one batch and 2048 queries vs all 4096 keys; outputs are disjoint slices.

Design notes (cost-model driven):
 - All matmuls run in f16 (1 PE cycle/output-column vs 4 for fp32).
 - S is computed TRANSPOSED (keys on partitions, queries free), 4 heads packed
   onto PE row strips via tile_position (32h, 0); each head's [128, 512] S^T
   needs its own PSUM bank (probed HW constraint for concurrent row strips),
   so heads go in pairs to 2-bank tiles [128, 2, 512].
 - exp is split across TWO engines: ScalarE computes exact exp -> f16; DVE
   computes a Schraudolph approximation (round(S*A+B) as int16 IS the f16
   bit pattern of exp(S*SCALE)); split ratio balances the two engines.
   Softmax renormalization absorbs the ~2% approximation noise.
 - PV runs TRANSPOSED as well: A^T[q,d] = sum_j P^T[j,q]^T v^T[j,d], with the
   512-wide P^T chunk as the STATIONARY operand and the 33-wide v chunk as
   the MOVING operand, accumulating over the 32 key chunks. vT carries an
   extra ones column so A^T column 32 is the softmax denominator -- a
   per-partition scalar, normalized with one reciprocal + broadcast multiply.
   Probed HW constraint: only one OPEN accumulation group per PSUM bank, so
   the 16 groups (4 q-subchunks x 4 heads) run as a sequential tail per
   query block, software-pipelined against the next block's S^T/exp stream
   (the P^T tiles of a block stay resident in SBUF: 64 tiles + slack).
 - an^T -> an via DMA-transpose (16x128 xbar tiles, f16), then a plain
   [c,q] = Wout^T.T @ an out-projection + bias, DMA'd out per [128, 512].
"""

import numpy as np
import sys

for _p in ("/opt/trn_rl_repo", "/opt/pypackages"):
    if _p not in sys.path:
        sys.path.append(_p)

import ml_dtypes
import concourse.bass as bass
import concourse.tile as tile
from concourse import bacc, mybir
from concourse.tile import add_dep_helper
from concourse.bass_utils import run_bass_kernel_spmd

f32 = mybir.dt.float32
f16 = mybir.dt.float16
i16 = mybir.dt.int16

B = 4
C = 256
N = 4096          # h*w = 64*64 key positions
NQ = 2048         # queries per core (half batch)
HEADS = 4
DH = 32
INNER = 128
SCALE = DH ** -0.5

QB = 512          # query block (free dim of S^T tiles)
NQB = NQ // QB    # 4
JT = 128          # key tile (partition dim of S^T tiles)
NJT = N // JT     # 32

PT_BUFS = 82   # P^T slots: 64 resident + next-block growth + xkv staging

# Schraudolph f16 exp: int16(round(S*A_EXP + B_EXP)) bitcast to f16
A_EXP = SCALE * 1024.0 / float(np.log(2.0))
B_EXP = 15360.0 - 55.0

# ScalarE : DVE exp tile split (261:256 scaled) balancing both engines
ACT_FRAC = 143.0 / 256.0


def _use_act(t):
    r = ACT_FRAC
    return int((t + 1) * r) - int(t * r) == 1


def build_nc():
    nc = bacc.Bacc()

    xkv_d = nc.dram_tensor("xkv", [C, N], f16, kind="ExternalInput")
    wqT_d = nc.dram_tensor("wqT", [C, INNER], f16, kind="ExternalInput")
    wkT_d = nc.dram_tensor("wkT", [C, INNER], f16, kind="ExternalInput")
    wvT_d = nc.dram_tensor("wvT", [C, INNER], f16, kind="ExternalInput")
    woT_d = nc.dram_tensor("woT", [INNER, C], f16, kind="ExternalInput")
    biasT_d = nc.dram_tensor("biasT", [128, 2], f32, kind="ExternalInput")
    out_d = nc.dram_tensor("out", [C, NQ], f32, kind="ExternalOutput")

    # One program for all 8 cores: the host passes xkv ROLLED so this core's
    # queries sit in columns 0:NQ. Key order is shared by k and v (both come
    # from the same rolled xkv), and softmax sums are order-invariant.
    q0 = 0

    with tile.TileContext(nc) as tc:
        import contextlib

        ctx = contextlib.ExitStack()
        with ctx:
            big = ctx.enter_context(tc.tile_pool(name="big", bufs=1))
            wk = ctx.enter_context(tc.tile_pool(name="wk", bufs=2))
            ptp = ctx.enter_context(tc.tile_pool(name="ptp", bufs=PT_BUFS))
            ps_st = ctx.enter_context(tc.tile_pool(name="ps_st", bufs=4, space="PSUM"))

            # ---- constants / weights ----
            wqT_sb = big.tile([128, 2, INNER], f16)   # [c_part, c_chunk, inner]
            wkT_sb = big.tile([128, 2, INNER], f16)
            wvT_sb = big.tile([128, 2, INNER], f16)
            woT_sb = big.tile([128, C], f16)          # [inner, c]
            bias_sb = big.tile([128, 2], f32)
            wqT_v = wqT_d.rearrange("(cc p) i -> p cc i", cc=2)
            wkT_v = wkT_d.rearrange("(cc p) i -> p cc i", cc=2)
            wvT_v = wvT_d.rearrange("(cc p) i -> p cc i", cc=2)
            xkv_v = xkv_d.rearrange("(cc p) n -> p cc n", cc=2)
            nc.sync.dma_start(out=wqT_sb[:], in_=wqT_v)
            nc.sync.dma_start(out=wkT_sb[:], in_=wkT_v)

            # ---- activations in: xkv lives in recyclable pt-pool slots ----
            # column-block tiles [128, 2(cc), 512]; slot is recycled into the
            # P^T pool once the projections for that block are done.
            xkvt = []
            for t in range(N // 512):
                xt = ptp.tile([128, 2, 512], f16, tag="pt", name="xkvt")
                nc.sync.dma_start(out=xt[:], in_=xkv_v[:, :, 512 * t:512 * (t + 1)])
                xkvt.append(xt)
                if t == 1:
                    nc.sync.dma_start(out=wvT_sb[:], in_=wvT_v)
            nc.sync.dma_start(out=woT_sb[:], in_=woT_d[:])
            nc.sync.dma_start(out=bias_sb[:], in_=biasT_d[:])

            k_sb = big.tile([128, N], f16)     # [inner, n]
            q_sb = big.tile([128, NQ], f16)    # [inner, nq]
            # v^T chunks + ones col: [j0, (jtile, head), 33]; col 32 = 1.0
            vT3 = big.tile([128, NJT * HEADS, DH + 1], f16)
            nc.gpsimd.memset(vT3[:, :, 32:33], 1.0)

            # ---- projections (emitted JIT inside qb0's J-loop) ----
            def emit_kproj(t):
                kp = ps_st.tile([128, 512], f32, tag="st", name="kp")
                for cc in range(2):
                    nc.tensor.matmul(
                        out=kp[:],
                        lhsT=wkT_sb[:, cc, :],
                        rhs=xkvt[t][:, cc, :],
                        start=(cc == 0), stop=(cc == 1),
                    )
                nc.scalar.copy(out=k_sb[:, 512 * t:512 * (t + 1)], in_=kp[:])

            def emit_qproj(t):
                qp = ps_st.tile([128, 512], f32, tag="st", name="qp")
                for cc in range(2):
                    nc.tensor.matmul(
                        out=qp[:],
                        lhsT=wqT_sb[:, cc, :],
                        rhs=xkvt[t][:, cc, :],
                        start=(cc == 0), stop=(cc == 1),
                    )
                nc.vector.tensor_copy(out=q_sb[:, 512 * t:512 * (t + 1)], in_=qp[:])

            def emit_vproj(t):
                # vT[n, inner] = x^T @ Wv^T, 128-row tiles of n
                vp = ps_st.tile([128, 4, 128], f32, tag="st", name="vp")
                for t2 in range(4):
                    for cc in range(2):
                        nc.tensor.matmul(
                            out=vp[:, t2, :],
                            lhsT=xkvt[t][:, cc, 128 * t2:128 * (t2 + 1)],
                            rhs=wvT_sb[:, cc, :],
                            start=(cc == 0), stop=(cc == 1),
                        )
                src = vp.rearrange("p t (h d) -> p (t h) d", d=DH)
                nc.vector.tensor_copy(
                    out=vT3[:, 16 * t:16 * (t + 1), 0:DH], in_=src
                )

            # ---- attention ----
            pt_tiles = {}     # (qb, J, p) -> pt AP
            exp_idx = [0]

            def emit_j(qb, J):
                for p in range(2):
                    st = ps_st.tile([128, 2, QB], f32, tag="st", name="st")
                    for hh in range(2):
                        h = 2 * p + hh
                        nc.tensor.matmul(
                            out=st[:, hh, :],
                            lhsT=k_sb[32 * h:32 * (h + 1), JT * J:JT * (J + 1)],
                            rhs=q_sb[32 * h:32 * (h + 1), QB * qb:QB * (qb + 1)],
                            start=True, stop=True,
                            tile_position=(32 * h, 0),
                        )
                    pt = ptp.tile([128, 2, QB], f16, tag="pt", name="pt")
                    pt_tiles[(qb, J, p)] = pt
                    t = exp_idx[0]
                    exp_idx[0] += 1
                    if _use_act(t):
                        nc.scalar.activation(
                            out=pt[:], in_=st[:],
                            func=mybir.ActivationFunctionType.Exp,
                            scale=SCALE,
                        )
                    else:
                        nc.vector.tensor_scalar(
                            out=pt.bitcast(i16)[:], in0=st[:],
                            scalar1=A_EXP, scalar2=B_EXP,
                            op0=mybir.AluOpType.mult, op1=mybir.AluOpType.add,
                        )

            # PV^T group order per acc bank: h-pairs first so each pt pair-
            # tile's last reader comes early and its slot recycles sooner.
            GORDER = [(0, 0), (0, 1), (1, 0), (1, 1), (0, 2), (0, 3), (1, 2), (1, 3)]

            def start_tail(qb):
                # all four accumulators in ONE 2-bank slot: bank b holds
                # q-subchunks (2b, 2b+1); one open group per bank still holds
                acc = ps_st.tile(
                    [128, 2, 2 * HEADS * (DH + 1)], f32,
                    padded_shape=[128, 2, 512], tag="st", name="acc",
                )
                av = acc.rearrange("p b (i h d) -> p b i h d", i=2, h=HEADS)
                return {"av": av, "prev": [None, None], "step": 0}

            def emit_pv_group(qb, state, bank, s):
                # one accumulation group (32 matmuls) on acc bank `bank`
                av = state["av"]
                ii, h = GORDER[s]
                i = 2 * bank + ii
                p, hh = h // 2, h % 2
                out_ap = av[:, bank, ii, h, :]
                prev = state["prev"][bank]
                for J in range(NJT):
                    mm = nc.tensor.matmul(
                        out=out_ap,
                        lhsT=pt_tiles[(qb, J, p)][:, hh, 128 * i:128 * (i + 1)],
                        rhs=vT3[:, HEADS * J + h, :],
                        start=(J == 0), stop=(J == NJT - 1),
                        skip_group_check=True,
                    )
                    if prev is not None:
                        add_dep_helper(mm.ins, prev.ins, sync=False, reason="pv order")
                    prev = mm
                state["prev"][bank] = prev

            out_v = out_d.rearrange("(cb p) n -> p cb n", cb=2)

            def finish_half(qb, an, half, on_act=False):
                # out projection + bias + store for q-subchunks 2h, 2h+1
                i0 = 2 * half
                rhs = an[:, i0:i0 + 2, :].rearrange("p b q -> p (b q)")
                op = ps_st.tile([128, 2, 256], f32, tag="st", name="op")
                for cb in range(2):
                    nc.tensor.matmul(
                        out=op[:, cb, :],
                        lhsT=woT_sb[:, 128 * cb:128 * (cb + 1)],
                        rhs=rhs,
                        start=True, stop=True,
                    )
                ob = wk.tile([128, 2, 256], f32, tag="ob", name="ob")
                if on_act:
                    # end-of-program: ScalarE is idle, DVE may not be
                    for cb in range(2):
                        nc.scalar.add(
                            out=ob[:, cb, :], in_=op[:, cb, :],
                            add=bias_sb[:, cb:cb + 1],
                        )
                else:
                    nc.vector.tensor_tensor(
                        out=ob[:], in0=op[:],
                        in1=bias_sb.unsqueeze(2).broadcast_to((128, 2, 256)),
                        op=mybir.AluOpType.add,
                    )
                c0 = QB * qb + 256 * half
                eng = nc.scalar if on_act else nc.sync
                eng.dma_start(out=out_v[:, :, c0:c0 + 256], in_=ob[:])

            def norm_bank(qb, state, bank, anT):
                # normalize: an^T[q, i, h, d] = A^T[q,i,h,d] / A^T[q,i,h,32]
                av = state["av"][:, bank]
                rcp = wk.tile([128, 2, 4], f32, tag="rcp", name="rcp")
                nc.vector.reciprocal(out=rcp[:], in_=av[:, :, :, DH])
                nc.vector.tensor_mul(
                    out=anT[:, 2 * bank:2 * bank + 2],
                    in0=av[:, :, :, 0:DH],
                    in1=rcp.unsqueeze(3).broadcast_to((128, 2, 4, DH)),
                )

            def finish_tail(qb, state):
                anT = wk.tile([128, 4, 4, DH], f16, tag="anT", name="anT")
                norm_bank(qb, state, 0, anT)
                norm_bank(qb, state, 1, anT)
                # batched DMA transpose an^T -> an[inner, i, q] (4 blocks)
                an = wk.tile([128, 4, 128], f16, tag="an", name="an")
                nc.sync.dma_start_transpose(
                    out=an[:], in_=anT.rearrange("q i h d -> q (i h d)")
                )
                finish_half(qb, an, 0)
                finish_half(qb, an, 1)

            # ---- main emission ----
            # qb0 carries the JIT projections; tails of qb spread across the
            # first J's of qb+1 (2 PV^T groups per J over J=2..9, finishers
            # at J=10); the last tail runs after the final J-loop.
            tail_state = None
            tail_qb = None
            emit_qproj(0)
            emit_qproj(1)
            emit_kproj(0)
            for qb in range(NQB):
                for J in range(NJT):
                    if qb == 0:
                        # JIT projections: k tile (J//4 + prefetch), q, v
                        if J % 4 == 2 and J // 4 + 1 < 8:
                            emit_kproj(J // 4 + 1)
                        if J == 0:
                            emit_qproj(2)
                            emit_qproj(3)
                        if J % 4 == 1:
                            emit_vproj(J // 4)
                    emit_j(qb, J)
                    if qb > 0:
                        if J == 1:
                            tail_state = start_tail(qb - 1)
                            tail_qb = qb - 1
                        elif 2 <= J <= 9:
                            emit_pv_group(tail_qb, tail_state, 0, J - 2)
                            emit_pv_group(tail_qb, tail_state, 1, J - 2)
                        elif J == 10:
                            finish_tail(tail_qb, tail_state)
                if qb == NQB - 1:
                    # final tail: bank A fully first so its normalize/
                    # transpose/out-proj overlap bank B's PV^T grind
                    st8 = start_tail(qb)
                    anT = wk.tile([128, 4, 4, DH], f16, tag="anT", name="anT")
                    an = wk.tile([128, 4, 128], f16, tag="an", name="an")
                    for s in range(8):
                        emit_pv_group(qb, st8, 0, s)
                    norm_bank(qb, st8, 0, anT)
                    nc.scalar.dma_start_transpose(
                        out=an[:, 0:2, :],
                        in_=anT[:, 0:2].rearrange("q i h d -> q (i h d)"),
                    )
                    for s in range(8):
                        emit_pv_group(qb, st8, 1, s)
                    norm_bank(qb, st8, 1, anT)
                    nc.scalar.dma_start_transpose(
                        out=an[:, 2:4, :],
                        in_=anT[:, 2:4].rearrange("q i h d -> q (i h d)"),
                    )
                    finish_half(qb, an, 0, on_act=True)
                    finish_half(qb, an, 1, on_act=True)

    nc.compile()
    return nc


_NC_CACHE = []


def _get_nc():
    if not _NC_CACHE:
        _NC_CACHE.append(build_nc())
    return _NC_CACHE[0]


def _make_in_maps(x, Wq, Wk, Wv, Wout, bout):
    bfl = np.float16
    xf = np.asarray(x, dtype=np.float32).reshape(B, C, N)
    wqT = np.ascontiguousarray(np.asarray(Wq, np.float32).T).astype(bfl)
    wkT = np.ascontiguousarray(np.asarray(Wk, np.float32).T).astype(bfl)
    wvT = np.ascontiguousarray(np.asarray(Wv, np.float32).T).astype(bfl)
    woT = np.ascontiguousarray(np.asarray(Wout, np.float32).T).astype(bfl)
    biasT = np.ascontiguousarray(
        np.asarray(bout, np.float32).reshape(2, 128).T
    ).astype(np.float32)
    in_maps = []
    for core in range(8):
        b, half = core // 2, core % 2
        q0 = half * NQ
        # roll keys so this core's queries occupy columns 0:NQ; key order is
        # shared by k and v so softmax/PV are unaffected.
        xroll = np.roll(xf[b], -q0, axis=1) if q0 else xf[b]
        in_maps.append({
            "xkv": np.ascontiguousarray(xroll).astype(bfl),
            "wqT": wqT, "wkT": wkT, "wvT": wvT,
            "woT": woT, "biasT": biasT,
        })
    return in_maps


def kernel(x, Wq, Wk, Wv, Wout, bout):
    nc = _get_nc()
    in_maps = _make_in_maps(x, Wq, Wk, Wv, Wout, bout)
    res = run_bass_kernel_spmd(nc, in_maps, core_ids=list(range(8)))
    out = np.empty((B, C, N), dtype=np.float32)
    for core in range(8):
        b, half = core // 2, core % 2
        q0 = half * NQ
        out[b][:, q0:q0 + NQ] = res.results[core]["out"]
    return out.reshape(B, C, 64, 64)


# revision 20
# speedup vs baseline: 1.0517x; 1.0517x over previous
"""Trainium2 Bass kernel for 4-head spatial self-attention (f16 pipeline).

Computation (per batch b):
    xf = x[b] reshaped [C=256, n=4096]
    q/k/v = Wq/Wk/Wv @ xf            -> [128, n]   (rows = 4 heads x 32 dims)
    S_h   = (q_h^T k_h) * 32^-0.5    -> [n, n] per head
    P     = exp(S)   (softmax without max-subtraction: logits are O(6))
    A_h   = P_h^T-normalized @ v_h   -> [n, 32]
    out   = Wout @ A + bout          -> [C, n]

Sharding: 8 cores = 4 batches x 2 query-halves. Each core handles all 4 heads
for one batch and 2048 queries vs all 4096 keys; outputs are disjoint slices.

Design notes (cost-model driven):
 - All matmuls run in f16 (1 PE cycle/output-column vs 4 for fp32).
 - S is computed TRANSPOSED (keys on partitions, queries free), 4 heads packed
   onto PE row strips via tile_position (32h, 0); each head's [128, 512] S^T
   needs its own PSUM bank (probed HW constraint for concurrent row strips),
   so heads go in pairs to 2-bank tiles [128, 2, 512].
 - exp is split across TWO engines: ScalarE computes exact exp -> f16; DVE
   computes a Schraudolph approximation (round(S*A+B) as int16 IS the f16
   bit pattern of exp(S*SCALE)); split ratio balances the two engines.
   Softmax renormalization absorbs the ~2% approximation noise.
 - PV runs TRANSPOSED as well: A^T[q,d] = sum_j P^T[j,q]^T v^T[j,d], with the
   512-wide P^T chunk as the STATIONARY operand and the 33-wide v chunk as
   the MOVING operand, accumulating over the 32 key chunks. vT carries an
   extra ones column so A^T column 32 is the softmax denominator -- a
   per-partition scalar, normalized with one reciprocal + broadcast multiply.
   Probed HW constraint: only one OPEN accumulation group per PSUM bank, so
   the 16 groups (4 q-subchunks x 4 heads) run as a sequential tail per
   query block, software-pipelined against the next block's S^T/exp stream
   (the P^T tiles of a block stay resident in SBUF: 64 tiles + slack).
 - an^T -> an via DMA-transpose (16x128 xbar tiles, f16), then a plain
   [c,q] = Wout^T.T @ an out-projection + bias, DMA'd out per [128, 512].
"""

import numpy as np
import sys

for _p in ("/opt/trn_rl_repo", "/opt/pypackages"):
    if _p not in sys.path:
        sys.path.append(_p)

import ml_dtypes
import concourse.bass as bass
import concourse.tile as tile
from concourse import bacc, mybir
from concourse.tile import add_dep_helper
from concourse.bass_utils import run_bass_kernel_spmd

f32 = mybir.dt.float32
f16 = mybir.dt.float16
i16 = mybir.dt.int16

B = 4
C = 256
N = 4096          # h*w = 64*64 key positions
NQ = 2048         # queries per core (half batch)
HEADS = 4
DH = 32
INNER = 128
SCALE = DH ** -0.5

QB = 512          # query block (free dim of S^T tiles)
NQB = NQ // QB    # 4
JT = 128          # key tile (partition dim of S^T tiles)
NJT = N // JT     # 32

PT_BUFS = 82   # P^T slots: 64 resident + next-block growth + xkv staging

# Schraudolph f16 exp: int16(round(S*A_EXP + B_EXP)) bitcast to f16
A_EXP = SCALE * 1024.0 / float(np.log(2.0))
B_EXP = 15360.0 - 55.0

# ScalarE : DVE exp tile split (261:256 scaled) balancing both engines
ACT_FRAC = 143.0 / 256.0


def _use_act(t):
    r = ACT_FRAC
    return int((t + 1) * r) - int(t * r) == 1


def build_nc():
    nc = bacc.Bacc()

    xkv_d = nc.dram_tensor("xkv", [C, N], f16, kind="ExternalInput")
    wqT_d = nc.dram_tensor("wqT", [C, INNER], f16, kind="ExternalInput")
    wkT_d = nc.dram_tensor("wkT", [C, INNER], f16, kind="ExternalInput")
    wvT_d = nc.dram_tensor("wvT", [C, INNER], f16, kind="ExternalInput")
    woT_d = nc.dram_tensor("woT", [INNER, C], f16, kind="ExternalInput")
    biasT_d = nc.dram_tensor("biasT", [128, 2], f32, kind="ExternalInput")
    out_d = nc.dram_tensor("out", [C, NQ], f32, kind="ExternalOutput")

    # One program for all 8 cores: the host passes xkv ROLLED so this core's
    # queries sit in columns 0:NQ. Key order is shared by k and v (both come
    # from the same rolled xkv), and softmax sums are order-invariant.
    q0 = 0

    with tile.TileContext(nc) as tc:
        import contextlib

        ctx = contextlib.ExitStack()
        with ctx:
            big = ctx.enter_context(tc.tile_pool(name="big", bufs=1))
            wk = ctx.enter_context(tc.tile_pool(name="wk", bufs=2))
            ptp = ctx.enter_context(tc.tile_pool(name="ptp", bufs=PT_BUFS))
            ps_st = ctx.enter_context(tc.tile_pool(name="ps_st", bufs=3, space="PSUM"))

            # ---- constants / weights ----
            wqT_sb = big.tile([128, 2, INNER], f16)   # [c_part, c_chunk, inner]
            wkT_sb = big.tile([128, 2, INNER], f16)
            wvT_sb = big.tile([128, 2, INNER], f16)
            woT_sb = big.tile([128, C], f16)          # [inner, c]
            bias_sb = big.tile([128, 2], f32)
            wqT_v = wqT_d.rearrange("(cc p) i -> p cc i", cc=2)
            wkT_v = wkT_d.rearrange("(cc p) i -> p cc i", cc=2)
            wvT_v = wvT_d.rearrange("(cc p) i -> p cc i", cc=2)
            xkv_v = xkv_d.rearrange("(cc p) n -> p cc n", cc=2)
            nc.sync.dma_start(out=wqT_sb[:], in_=wqT_v)
            nc.sync.dma_start(out=wkT_sb[:], in_=wkT_v)

            # ---- activations in: xkv lives in recyclable pt-pool slots ----
            # column-block tiles [128, 2(cc), 512]; slot is recycled into the
            # P^T pool once the projections for that block are done.
            xkvt = []
            for t in range(N // 512):
                xt = ptp.tile([128, 2, 512], f16, tag="pt", name="xkvt")
                nc.sync.dma_start(out=xt[:], in_=xkv_v[:, :, 512 * t:512 * (t + 1)])
                xkvt.append(xt)
                if t == 1:
                    nc.sync.dma_start(out=wvT_sb[:], in_=wvT_v)
            nc.sync.dma_start(out=woT_sb[:], in_=woT_d[:])
            nc.sync.dma_start(out=bias_sb[:], in_=biasT_d[:])

            k_sb = big.tile([128, N], f16)     # [inner, n]
            q_sb = big.tile([128, NQ], f16)    # [inner, nq]
            # v^T chunks + ones col: [j0, (jtile, head), 33]; col 32 = 1.0
            vT3 = big.tile([128, NJT * HEADS, DH + 1], f16)
            nc.gpsimd.memset(vT3[:, :, 32:33], 1.0)

            # ---- projections (emitted JIT inside qb0's J-loop) ----
            def emit_kproj(t):
                kp = ps_st.tile([128, 512], f32, tag="st", name="kp")
                for cc in range(2):
                    nc.tensor.matmul(
                        out=kp[:],
                        lhsT=wkT_sb[:, cc, :],
                        rhs=xkvt[t][:, cc, :],
                        start=(cc == 0), stop=(cc == 1),
                    )
                nc.scalar.copy(out=k_sb[:, 512 * t:512 * (t + 1)], in_=kp[:])

            def emit_qproj(t):
                qp = ps_st.tile([128, 512], f32, tag="st", name="qp")
                for cc in range(2):
                    nc.tensor.matmul(
                        out=qp[:],
                        lhsT=wqT_sb[:, cc, :],
                        rhs=xkvt[t][:, cc, :],
                        start=(cc == 0), stop=(cc == 1),
                    )
                nc.vector.tensor_copy(out=q_sb[:, 512 * t:512 * (t + 1)], in_=qp[:])

            def emit_vproj(t):
                # vT[n, inner] = x^T @ Wv^T, 128-row tiles of n
                vp = ps_st.tile([128, 4, 128], f32, tag="st", name="vp")
                for t2 in range(4):
                    for cc in range(2):
                        nc.tensor.matmul(
                            out=vp[:, t2, :],
                            lhsT=xkvt[t][:, cc, 128 * t2:128 * (t2 + 1)],
                            rhs=wvT_sb[:, cc, :],
                            start=(cc == 0), stop=(cc == 1),
                        )
                src = vp.rearrange("p t (h d) -> p (t h) d", d=DH)
                nc.vector.tensor_copy(
                    out=vT3[:, 16 * t:16 * (t + 1), 0:DH], in_=src
                )

            # ---- attention ----
            pt_tiles = {}     # (qb, J, p) -> pt AP
            exp_idx = [0]

            def emit_j(qb, J):
                for p in range(2):
                    st = ps_st.tile([128, 2, QB], f32, tag="st", name="st")
                    for hh in range(2):
                        h = 2 * p + hh
                        nc.tensor.matmul(
                            out=st[:, hh, :],
                            lhsT=k_sb[32 * h:32 * (h + 1), JT * J:JT * (J + 1)],
                            rhs=q_sb[32 * h:32 * (h + 1), QB * qb:QB * (qb + 1)],
                            start=True, stop=True,
                            tile_position=(32 * h, 0),
                        )
                    pt = ptp.tile([128, 2, QB], f16, tag="pt", name="pt")
                    pt_tiles[(qb, J, p)] = pt
                    t = exp_idx[0]
                    exp_idx[0] += 1
                    if _use_act(t):
                        nc.scalar.activation(
                            out=pt[:], in_=st[:],
                            func=mybir.ActivationFunctionType.Exp,
                            scale=SCALE,
                        )
                    else:
                        nc.vector.tensor_scalar(
                            out=pt.bitcast(i16)[:], in0=st[:],
                            scalar1=A_EXP, scalar2=B_EXP,
                            op0=mybir.AluOpType.mult, op1=mybir.AluOpType.add,
                        )

            # PV^T group order per acc bank: h-pairs first so each pt pair-
            # tile's last reader comes early and its slot recycles sooner.
            GORDER = [(0, 0), (0, 1), (1, 0), (1, 1), (0, 2), (0, 3), (1, 2), (1, 3)]

            def start_tail(qb):
                # all four accumulators in ONE 2-bank slot: bank b holds
                # q-subchunks (2b, 2b+1); one open group per bank still holds
                acc = ps_st.tile(
                    [128, 2, 2 * HEADS * (DH + 1)], f32,
                    padded_shape=[128, 2, 512], tag="acc", bufs=1, name="acc",
                )
                av = acc.rearrange("p b (i h d) -> p b i h d", i=2, h=HEADS)
                return {"av": av, "prev": [None, None], "step": 0}

            def emit_pv_group(qb, state, bank, s):
                # one accumulation group (32 matmuls) on acc bank `bank`
                av = state["av"]
                ii, h = GORDER[s]
                i = 2 * bank + ii
                p, hh = h // 2, h % 2
                out_ap = av[:, bank, ii, h, :]
                prev = state["prev"][bank]
                for J in range(NJT):
                    mm = nc.tensor.matmul(
                        out=out_ap,
                        lhsT=pt_tiles[(qb, J, p)][:, hh, 128 * i:128 * (i + 1)],
                        rhs=vT3[:, HEADS * J + h, :],
                        start=(J == 0), stop=(J == NJT - 1),
                        skip_group_check=True,
                    )
                    if prev is not None:
                        add_dep_helper(mm.ins, prev.ins, sync=False, reason="pv order")
                    prev = mm
                state["prev"][bank] = prev

            out_v = out_d.rearrange("(cb p) n -> p cb n", cb=2)

            def finish_half(qb, an, half, on_act=False):
                # out projection + bias + store for q-subchunks 2h, 2h+1
                i0 = 2 * half
                rhs = an[:, i0:i0 + 2, :].rearrange("p b q -> p (b q)")
                op = ps_st.tile([128, 2, 256], f32, tag="acc", bufs=1, name="op")
                for cb in range(2):
                    nc.tensor.matmul(
                        out=op[:, cb, :],
                        lhsT=woT_sb[:, 128 * cb:128 * (cb + 1)],
                        rhs=rhs,
                        start=True, stop=True,
                    )
                ob = wk.tile([128, 2, 256], f32, tag="ob", name="ob")
                if on_act:
                    # end-of-program: ScalarE is idle, DVE may not be
                    for cb in range(2):
                        nc.scalar.add(
                            out=ob[:, cb, :], in_=op[:, cb, :],
                            add=bias_sb[:, cb:cb + 1],
                        )
                else:
                    nc.vector.tensor_tensor(
                        out=ob[:], in0=op[:],
                        in1=bias_sb.unsqueeze(2).broadcast_to((128, 2, 256)),
                        op=mybir.AluOpType.add,
                    )
                c0 = QB * qb + 256 * half
                eng = nc.scalar if on_act else nc.sync
                eng.dma_start(out=out_v[:, :, c0:c0 + 256], in_=ob[:])

            def norm_bank(qb, state, bank, anT):
                # normalize: an^T[q, i, h, d] = A^T[q,i,h,d] / A^T[q,i,h,32]
                av = state["av"][:, bank]
                rcp = wk.tile([128, 2, 4], f32, tag="rcp", name="rcp")
                nc.vector.reciprocal(out=rcp[:], in_=av[:, :, :, DH])
                nc.vector.tensor_mul(
                    out=anT[:, 2 * bank:2 * bank + 2],
                    in0=av[:, :, :, 0:DH],
                    in1=rcp.unsqueeze(3).broadcast_to((128, 2, 4, DH)),
                )

            def finish_tail(qb, state):
                anT = wk.tile([128, 4, 4, DH], f16, tag="anT", name="anT")
                norm_bank(qb, state, 0, anT)
                norm_bank(qb, state, 1, anT)
                # batched DMA transpose an^T -> an[inner, i, q] (4 blocks)
                an = wk.tile([128, 4, 128], f16, tag="an", name="an")
                nc.sync.dma_start_transpose(
                    out=an[:], in_=anT.rearrange("q i h d -> q (i h d)")
                )
                finish_half(qb, an, 0)
                finish_half(qb, an, 1)

            # ---- main emission ----
            # qb0 carries the JIT projections; tails of qb spread across the
            # first J's of qb+1 (2 PV^T groups per J over J=2..9, finishers
            # at J=10); the last tail runs after the final J-loop.
            tail_state = None
            tail_qb = None
            emit_qproj(0)
            emit_qproj(1)
            emit_kproj(0)
            for qb in range(NQB):
                for J in range(NJT):
                    if qb == 0:
                        # JIT projections: k tile (J//4 + prefetch), q, v
                        if J % 4 == 2 and J // 4 + 1 < 8:
                            emit_kproj(J // 4 + 1)
                        if J == 0:
                            emit_qproj(2)
                            emit_qproj(3)
                        if J % 4 == 1:
                            emit_vproj(J // 4)
                    emit_j(qb, J)
                    if qb > 0:
                        if J == 1:
                            tail_state = start_tail(qb - 1)
                            tail_qb = qb - 1
                        elif 2 <= J <= 9:
                            emit_pv_group(tail_qb, tail_state, 0, J - 2)
                            emit_pv_group(tail_qb, tail_state, 1, J - 2)
                        elif J == 10:
                            finish_tail(tail_qb, tail_state)
                if qb == NQB - 1:
                    # final tail: bank A fully first so its normalize/
                    # transpose/out-proj overlap bank B's PV^T grind
                    st8 = start_tail(qb)
                    anT = wk.tile([128, 4, 4, DH], f16, tag="anT", name="anT")
                    an = wk.tile([128, 4, 128], f16, tag="an", name="an")
                    for s in range(8):
                        emit_pv_group(qb, st8, 0, s)
                    norm_bank(qb, st8, 0, anT)
                    nc.scalar.dma_start_transpose(
                        out=an[:, 0:2, :],
                        in_=anT[:, 0:2].rearrange("q i h d -> q (i h d)"),
                    )
                    for s in range(8):
                        emit_pv_group(qb, st8, 1, s)
                    norm_bank(qb, st8, 1, anT)
                    nc.scalar.dma_start_transpose(
                        out=an[:, 2:4, :],
                        in_=anT[:, 2:4].rearrange("q i h d -> q (i h d)"),
                    )
                    finish_half(qb, an, 0, on_act=True)
                    finish_half(qb, an, 1, on_act=True)

    nc.compile()
    return nc


_NC_CACHE = []


def _get_nc():
    if not _NC_CACHE:
        _NC_CACHE.append(build_nc())
    return _NC_CACHE[0]


def _make_in_maps(x, Wq, Wk, Wv, Wout, bout):
    bfl = np.float16
    xf = np.asarray(x, dtype=np.float32).reshape(B, C, N)
    wqT = np.ascontiguousarray(np.asarray(Wq, np.float32).T).astype(bfl)
    wkT = np.ascontiguousarray(np.asarray(Wk, np.float32).T).astype(bfl)
    wvT = np.ascontiguousarray(np.asarray(Wv, np.float32).T).astype(bfl)
    woT = np.ascontiguousarray(np.asarray(Wout, np.float32).T).astype(bfl)
    biasT = np.ascontiguousarray(
        np.asarray(bout, np.float32).reshape(2, 128).T
    ).astype(np.float32)
    in_maps = []
    for core in range(8):
        b, half = core // 2, core % 2
        q0 = half * NQ
        # roll keys so this core's queries occupy columns 0:NQ; key order is
        # shared by k and v so softmax/PV are unaffected.
        xroll = np.roll(xf[b], -q0, axis=1) if q0 else xf[b]
        in_maps.append({
            "xkv": np.ascontiguousarray(xroll).astype(bfl),
            "wqT": wqT, "wkT": wkT, "wvT": wvT,
            "woT": woT, "biasT": biasT,
        })
    return in_maps


def kernel(x, Wq, Wk, Wv, Wout, bout):
    nc = _get_nc()
    in_maps = _make_in_maps(x, Wq, Wk, Wv, Wout, bout)
    res = run_bass_kernel_spmd(nc, in_maps, core_ids=list(range(8)))
    out = np.empty((B, C, N), dtype=np.float32)
    for core in range(8):
        b, half = core // 2, core % 2
        q0 = half * NQ
        out[b][:, q0:q0 + NQ] = res.results[core]["out"]
    return out.reshape(B, C, 64, 64)
